# revision 43
# baseline (speedup 1.0000x reference)
"""Trainium2 Bass kernel for the Context Encoder problem:

    ce  = c2e_weight[nodes]            # [N, 128] embedding gather
    h   = relu(ce @ w1.T + b1)         # [N, 128]
    out = relu(h @ w2.T + b2)          # [N, 128]

Strategy (8 NeuronCores, unique-row compaction):
  200000 node ids hit ~86.4k of the 100k vocab rows, so transforming
  each UNIQUE row once is cheaper than gathering per-node rows.  The
  host maps node positions to unique rows (out = T2c[inv]) as the
  unshard step.  Core i streams its host-pre-transposed (d-major)
  slice of the compacted table [128, ~10880] and computes
  T2 = relu(relu(win@w1.T+b1)@w2.T+b2) column-by-column.

  Perf structure (memory regime, ~26GB/s x 16 DMA engines/core):
  - bf16 on the wire and through the PE: halves HBM traffic; the warm
    PE streams 512-col bf16 matmuls every ~215ns with LDWEIGHTS hidden
    by its 64-deep reorder window, so w1/w2 alternation is free.  PSUM
    stays f32.
  - 1024-col groups (2 PSUM banks): relu+bias is ONE fused op per
    group per layer, alternated ACT/DVE (the only engines with PSUM
    ports; ~1.11/1.27us per op, which sets the compute pace).  Small
    first groups (128, 512) prime the pipeline; small last groups
    (512, 512) shorten the drain.
  - mm1 is emitted 3 groups ahead through a 4-deep PSUM rotation
    (8 banks), so the mm1 -> relu_h -> mm2 -> relu_o chain never
    starves the PE; mm2 overwrites its own group's PSUM tile (free
    once relu_h read it) - no WAR stalls.
  - ALL input chunks ride the Sync HWDGE queue (a queue's chunks
    transfer serially at full 16-engine rate, so in-order chunk
    completion is earliest), one chunk per group so a group's matmul
    never waits on a later group's data.  Output batches of ~2 groups
    flush on the otherwise-idle GpSimd queue as soon as their relu_o
    completes, so the out-stream overlaps the in-stream and compute
    window; the small tail batches are partition-split across both
    queues to shorten the critical tail.  Neither relu engine ever
    issues a DMA.
  - Zero-input dummy matmuls during the initial DMA wait heat the
    PE's HAM clock gate (cold = 1.2GHz, warm = 2.4GHz) so real
    matmuls run warm from the start.
"""

import sys

for _p in ("/opt/trn_rl_repo",):
    if _p not in sys.path:
        sys.path.insert(0, _p)

import ml_dtypes
import numpy as np

import concourse.bass as bass
import concourse.mybir as mybir
from concourse import bacc
from concourse.bass_utils import run_bass_kernel_spmd
from concourse.tile import TileContext

P = 128
D = 128
N_CORES = 8
VOCAB = 100000
RANGE = VOCAB // N_CORES   # full-table fallback: vocab rows per core
FBLOCKS = 98               # full-table fallback: 12544 rows/core
CBLOCKS = 85               # compacted (unique-rows) path: 10880 rows/core
CBLOCKS2 = 88              # compacted fallback if uniques don't fit 85
MMW = 512                  # matmul free width (1 PSUM bank)
GW = 1024                  # relu group width (2 PSUM banks)
HDR = 264                  # header cols prepended to chunk 0

BF16 = ml_dtypes.bfloat16


def _groups(cols):
    # [128, 512] primer groups, full 1024-col groups mid-stream, and
    # two ~half groups at the end to shorten the pipeline drain.
    rem = cols - 640
    k, r = divmod(rem, GW)
    tail = GW + r
    a = ((tail // 2) + P - 1) // P * P
    return [P, MMW] + [GW] * (k - 1) + [a, tail - a]


def _chunks(n):
    # input DMA chunks as group-ranges: singles for the three primer
    # groups (earliest possible compute start), then PAIRS.  Groups
    # arrive at ~1.0us/group with single-group chunks - exactly the
    # relu consumption pace, so every per-chunk ramp hiccup (~0.3us
    # desc-fetch penalty) becomes a pipeline bubble; pairing halves
    # the per-group overhead and builds ~0.8us of arrival slack.
    k = min(3, n)
    chunks = [(i, i + 1) for i in range(k)]
    i = k
    while i < n:
        j = min(i + 2, n)
        if n - j == 1:
            j = n
        chunks.append((i, j))
        i = j
    return chunks


def _batches(n):
    # output DMA batches as group-ranges: the first three groups
    # together, then two-group batches, then the last two groups as
    # separate small batches (they sit on the critical tail).
    batches = [(0, min(3, n))]
    i = batches[-1][1]
    while i < n - 3:
        batches.append((i, min(i + 2, n - 3)))
        i = batches[-1][1]
    for j in range(i, n):
        batches.append((j, j + 1))
    return batches


# groups whose BOTH relus run on ACT: the two PSUM-capable engines are
# unequal (ACT ~1.09ns/col vs DVE ~1.24ns/col), so strict alternation
# leaves ACT idle while DVE paces the pipeline; shifting one mid-stream
# group entirely to ACT balances their totals.
BOTH_ACT = {7}
N_WARM_MM = 6   # dummy matmuls that heat the PE HAM during the DMA wait
WARM_W = 512    # width of each dummy matmul


def build_nc(blocks):
    f32 = mybir.dt.float32
    bf16 = mybir.dt.bfloat16
    nc = bacc.Bacc("TRN2", target_bir_lowering=False, debug=False,
                   num_devices=N_CORES)

    cols = blocks * P
    groups = _groups(cols)
    goff = np.concatenate([[0], np.cumsum(groups)]).tolist()
    assert goff[-1] == cols, (goff[-1], cols)
    n_g = len(groups)
    chunks = _chunks(n_g)
    batches = _batches(n_g)

    tsl_t = nc.dram_tensor("tslice", [P, HDR + cols], bf16,
                           kind="ExternalInput").ap()
    out_t = nc.dram_tensor("out", [P, cols], bf16,
                           kind="ExternalOutput").ap()

    group_chunk = {}      # group -> chunk idx
    for ci, (a, b) in enumerate(chunks):
        for g in range(a, b):
            group_chunk[g] = ci

    group_batch = {}      # group -> (batch idx, last-in-batch?)
    for bi, (a, b) in enumerate(batches):
        for g in range(a, b):
            group_batch[g] = (bi, g == b - 1)

    with TileContext(nc) as tc:
        with (
            tc.tile_pool(name="const", bufs=1) as cpool,
            tc.tile_pool(name="winp", bufs=1) as gpool,
            tc.tile_pool(name="hT", bufs=6) as hpool,
            tc.tile_pool(name="outs", bufs=1) as opool,
            tc.tile_pool(name="psum", bufs=4, space="PSUM") as ppool,
        ):
            # pre-warm the Scalar engine's activation table (the lazy
            # ACT_TABLE_LOAD otherwise lands on the first relu's
            # critical path, costing ~1.3us)
            warm = cpool.tile([P, 1], f32, tag="warm")
            nc.vector.memset(warm[:], 0.0)
            nc.scalar.activation(warm[:], warm[:],
                                 mybir.ActivationFunctionType.Relu, bias=0.0)

            # zero tile feeding the PE warm-up matmuls below
            zdum = cpool.tile([P, WARM_W], bf16, tag="zdum")
            nc.vector.memset(zdum[:], 0.0)

            # pre-warm the GpSimd HWDGE queue: the first DMA on a
            # queue pays a ~0.6us one-time config; a 1-col dummy load
            # moves that off batch 0's flush path.
            scratch = cpool.tile([P, 1], bf16, tag="qwarm")
            nc.gpsimd.dma_start(out=scratch[:], in_=tsl_t[:, 0:1])

            # the whole input window fits in SBUF: queue every chunk's
            # DMA upfront on the Sync queue so chunks stream back-to-
            # back in consumption order at the full 16-engine rate.
            win_tiles = []
            for ci, (a, b) in enumerate(chunks):
                lo = goff[a] + (0 if ci == 0 else HDR)
                cw = goff[b] - goff[a] + (HDR if ci == 0 else 0)
                win = gpool.tile([P, cw], bf16, tag=f"win{ci}")
                nc.sync.dma_start(out=win[:], in_=tsl_t[:, lo : lo + cw])
                win_tiles.append(win)

            hdr = win_tiles[0]
            w1t_sb = hdr[:, 0:D]
            w2t_sb = hdr[:, D : 2 * D]
            b12_sb = hdr[:, 2 * D : 2 * D + 4].bitcast(f32)
            b1_sb = b12_sb[:, 0:1]
            b2_sb = b12_sb[:, 1:2]

            def win_ap(g):
                ci = group_chunk[g]
                a, _ = chunks[ci]
                off = goff[g] - goff[a] + (HDR if ci == 0 else 0)
                return win_tiles[ci][:, off : off + groups[g]]

            # one SBUF tile per output batch
            out_tiles = []
            for bi, (a, b) in enumerate(batches):
                bw = goff[b] - goff[a]
                out_tiles.append(opool.tile([P, bw], bf16,
                                            name=f"outsb{bi}",
                                            tag=f"out{bi}"))

            def relu_bias(out_ap, in_ap, bias_sb, on_act):
                if on_act:
                    nc.scalar.activation(out_ap, in_ap,
                                         mybir.ActivationFunctionType.Relu,
                                         bias=bias_sb)
                else:
                    nc.vector.tensor_scalar(
                        out=out_ap, in0=in_ap, scalar1=bias_sb,
                        scalar2=0.0, op0=mybir.AluOpType.add,
                        op1=mybir.AluOpType.max)

            ps = {}
            hT = {}

            def mm1(g):
                gw = groups[g]
                src = win_ap(g)
                t = ppool.tile([P, GW], f32, tag="ps")
                for s in range(0, gw, MMW):
                    w = min(MMW, gw - s)
                    nc.tensor.matmul(out=t[:, s : s + w], lhsT=w1t_sb,
                                     rhs=src[:, s : s + w],
                                     start=True, stop=True)
                ps[g] = t

            def rh(g):
                gw = groups[g]
                t = hpool.tile([P, GW], bf16, tag="hT")
                relu_bias(t[:, :gw], ps[g][:, :gw], b1_sb,
                          on_act=(g % 2 == 0 or g in BOTH_ACT))
                hT[g] = t

            def mm2(g):
                # overwrites the group's own h PSUM tile (free once
                # relu_h has read it)
                gw = groups[g]
                src = hT.pop(g)
                t = ps[g]
                for s in range(0, gw, MMW):
                    w = min(MMW, gw - s)
                    nc.tensor.matmul(out=t[:, s : s + w], lhsT=w2t_sb,
                                     rhs=src[:, s : s + w],
                                     start=True, stop=True)

            def ro(g):
                gw = groups[g]
                bi, last = group_batch[g]
                a, b = batches[bi]
                boff = goff[g] - goff[a]
                out_sb = out_tiles[bi]
                relu_bias(out_sb[:, boff : boff + gw], ps.pop(g)[:, :gw],
                          b2_sb, on_act=(g % 2 == 1 or g in BOTH_ACT))
                if last:
                    bw = goff[b] - goff[a]
                    bstart = goff[a]
                    if bi >= len(batches) - 4:
                        # tail batches: partition-split across both
                        # queues so their transfers and desc-gen run
                        # in parallel (they sit on the critical tail;
                        # Sync's input triggers are long done)
                        nc.gpsimd.dma_start(
                            out=out_t[0:64, bstart : bstart + bw],
                            in_=out_sb[0:64, :])
                        nc.sync.dma_start(
                            out=out_t[64:128, bstart : bstart + bw],
                            in_=out_sb[64:128, :])
                    else:
                        nc.gpsimd.dma_start(
                            out=out_t[:, bstart : bstart + bw],
                            in_=out_sb[:])

            # PE warm-up: the HAM clock gate keeps a cold PE at
            # 1.2GHz until it has seen ~3.4us of sustained activity.
            # Zero-input dummy matmuls during the otherwise-idle
            # chunk-0 DMA wait flip it to 2.4GHz right as real work
            # arrives (cold 512-col matmuls cost 630ns vs 215 warm).
            wps = ppool.tile([P, GW], f32, tag="ps")
            for _ in range(N_WARM_MM):
                nc.tensor.matmul(out=wps[:, 0:WARM_W], lhsT=zdum[:, 0:D],
                                 rhs=zdum[:], start=True, stop=True)

            # software pipeline: mm1 runs 3 groups ahead through the
            # 4-deep PSUM rotation so relu latency never starves the
            # PE or lets the DMA bus idle.
            LOOK = 3
            for g in range(min(LOOK, n_g)):
                mm1(g)
            if n_g:
                rh(0)
            for g in range(n_g):
                if g + 1 < n_g:
                    rh(g + 1)
                mm2(g)
                if g + LOOK < n_g:
                    mm1(g + LOOK)
                ro(g)

    nc.compile()
    return nc


_CACHED_NC = {}
LAST_RESULTS = None


def _get_nc(blocks):
    if blocks not in _CACHED_NC:
        _CACHED_NC[blocks] = build_nc(blocks)
    return _CACHED_NC[blocks]


def _run(in_maps):
    import os

    trace = os.environ.get("BASS_KERNEL_TRACE") == "1"
    if trace:
        try:  # tracing needs the NTFF hook; degrade silently without it
            import antenv.axon_hooks  # noqa: F401
        except ImportError:
            trace = False
    blocks = (in_maps[0]["tslice"].shape[1] - HDR) // P
    res = run_bass_kernel_spmd(_get_nc(blocks), in_maps,
                               core_ids=list(range(N_CORES)), trace=trace)
    global LAST_RESULTS
    LAST_RESULTS = res
    return res


def _make_header(w1, b1, w2, b2):
    wb = np.concatenate([w1.T, w2.T], axis=1).astype(BF16)    # [128, 256]
    b12 = np.stack([b1, b2], axis=1).astype(np.float32)       # [128, 2]
    b12_bf = b12.view(BF16)                                   # [128, 4] raw
    pad = np.zeros((P, HDR - 2 * D - 4), dtype=BF16)
    return np.concatenate([wb, b12_bf, pad], axis=1)          # [128, HDR]


def kernel(nodes, c2e_weight, w1, b1, w2, b2):
    nodes = np.asarray(nodes).astype(np.int64)
    c2e_weight = np.asarray(c2e_weight, dtype=np.float32)
    w1 = np.asarray(w1, dtype=np.float32)
    b1 = np.asarray(b1, dtype=np.float32)
    w2 = np.asarray(w2, dtype=np.float32)
    b2 = np.asarray(b2, dtype=np.float32)

    vocab = c2e_weight.shape[0]
    assert vocab == VOCAB, vocab

    tableT = np.ascontiguousarray(c2e_weight.T).astype(BF16)  # [128, V]
    header = _make_header(w1, b1, w2, b2)

    # Compacted path: only vocab rows actually referenced by `nodes`
    # are pushed through the MLP (~86.4% of the vocab for uniform
    # ids).  Falls back to a wider layout / transforming the full
    # table when the unique count doesn't fit.
    uniq, inv = np.unique(nodes, return_inverse=True)
    u = len(uniq)
    for cblocks in (CBLOCKS, CBLOCKS2):
        crows = cblocks * P
        if u > N_CORES * crows:
            continue
        tc = tableT[:, uniq]                              # [128, U] gather
        if u < N_CORES * crows:
            tc = np.concatenate(
                [tc, np.zeros((P, N_CORES * crows - u), dtype=BF16)],
                axis=1)
        in_maps = [{
            "tslice": np.ascontiguousarray(np.concatenate(
                [header, tc[:, i * crows : (i + 1) * crows]], axis=1)),
        } for i in range(N_CORES)]
        res = _run(in_maps)
        t2c = np.empty((u, D), dtype=np.float32)
        for i in range(N_CORES):
            lo = i * crows
            hi = min(lo + crows, u)
            if lo >= u:
                break
            dense = res.results[i]["out"]                 # [128, crows]
            t2c[lo:hi] = dense[:, : hi - lo].T
        return t2c[inv]

    # Full-table fallback: vocab-range sharding, T2[v] for every v
    rows = FBLOCKS * P
    starts = []
    in_maps = []
    for i in range(N_CORES):
        start = min(i * RANGE, vocab - rows)
        starts.append(start)
        in_maps.append({
            "tslice": np.ascontiguousarray(np.concatenate(
                [header, tableT[:, start : start + rows]], axis=1)),
        })
    res = _run(in_maps)
    t2 = np.empty((vocab, D), dtype=np.float32)
    for i in range(N_CORES):
        dense = res.results[i]["out"]                    # [128, rows] (k, r)
        lo = i * RANGE
        hi = min((i + 1) * RANGE, vocab)
        t2[lo:hi] = dense[:, lo - starts[i] : hi - starts[i]].T

    return t2[nodes]


# revision 45
# speedup vs baseline: 1.0138x; 1.0138x over previous
"""Trainium2 Bass kernel for the Context Encoder problem:

    ce  = c2e_weight[nodes]            # [N, 128] embedding gather
    h   = relu(ce @ w1.T + b1)         # [N, 128]
    out = relu(h @ w2.T + b2)          # [N, 128]

Strategy (8 NeuronCores, unique-row compaction):
  200000 node ids hit ~86.4k of the 100k vocab rows, so transforming
  each UNIQUE row once is cheaper than gathering per-node rows.  The
  host maps node positions to unique rows (out = T2c[inv]) as the
  unshard step.  Core i streams its host-pre-transposed (d-major)
  slice of the compacted table [128, ~10880] and computes
  T2 = relu(relu(win@w1.T+b1)@w2.T+b2) column-by-column.

  Perf structure (memory regime, ~26GB/s x 16 DMA engines/core):
  - bf16 on the wire and through the PE: halves HBM traffic; the warm
    PE streams 512-col bf16 matmuls every ~215ns with LDWEIGHTS hidden
    by its 64-deep reorder window, so w1/w2 alternation is free.  PSUM
    stays f32.
  - 1024-col groups (2 PSUM banks): relu+bias is ONE fused op per
    group per layer, alternated ACT/DVE (the only engines with PSUM
    ports; ~1.11/1.27us per op, which sets the compute pace).  Small
    first groups (128, 512) prime the pipeline; small last groups
    (512, 512) shorten the drain.
  - mm1 is emitted 3 groups ahead through a 4-deep PSUM rotation
    (8 banks), so the mm1 -> relu_h -> mm2 -> relu_o chain never
    starves the PE; mm2 overwrites its own group's PSUM tile (free
    once relu_h read it) - no WAR stalls.
  - ALL input chunks ride the Sync HWDGE queue (a queue's chunks
    transfer serially at full 16-engine rate, so in-order chunk
    completion is earliest), one chunk per group so a group's matmul
    never waits on a later group's data.  Output batches of ~2 groups
    flush on the otherwise-idle GpSimd queue as soon as their relu_o
    completes, so the out-stream overlaps the in-stream and compute
    window; the small tail batches are partition-split across both
    queues to shorten the critical tail.  Neither relu engine ever
    issues a DMA.
  - Zero-input dummy matmuls during the initial DMA wait heat the
    PE's HAM clock gate (cold = 1.2GHz, warm = 2.4GHz) so real
    matmuls run warm from the start.
"""

import sys

for _p in ("/opt/trn_rl_repo",):
    if _p not in sys.path:
        sys.path.insert(0, _p)

import ml_dtypes
import numpy as np

import concourse.bass as bass
import concourse.mybir as mybir
from concourse import bacc
from concourse.bass_utils import run_bass_kernel_spmd
from concourse.tile import TileContext

P = 128
D = 128
N_CORES = 8
VOCAB = 100000
RANGE = VOCAB // N_CORES   # full-table fallback: vocab rows per core
FBLOCKS = 98               # full-table fallback: 12544 rows/core
CBLOCKS = 85               # compacted (unique-rows) path: 10880 rows/core
CBLOCKS2 = 88              # compacted fallback if uniques don't fit 85
MMW = 512                  # matmul free width (1 PSUM bank)
GW = 1024                  # relu group width (2 PSUM banks)
HDR = 264                  # header cols prepended to chunk 0

BF16 = ml_dtypes.bfloat16


def _groups(cols):
    # [128, 512] primer groups, full 1024-col groups mid-stream, and
    # two ~half groups at the end to shorten the pipeline drain.
    rem = cols - 640
    k, r = divmod(rem, GW)
    tail = GW + r
    a = ((tail // 2) + P - 1) // P * P
    return [P, MMW] + [GW] * (k - 1) + [a, tail - a]


def _chunks(n):
    # input DMA chunks as group-ranges: singles for the three primer
    # groups (earliest possible compute start), then PAIRS.  Groups
    # arrive at ~1.0us/group with single-group chunks - exactly the
    # relu consumption pace, so every per-chunk ramp hiccup (~0.3us
    # desc-fetch penalty) becomes a pipeline bubble; pairing halves
    # the per-group overhead and builds ~0.8us of arrival slack.
    k = min(3, n)
    chunks = [(i, i + 1) for i in range(k)]
    i = k
    while i < n:
        j = min(i + 2, n)
        if n - j == 1:
            j = n
        chunks.append((i, j))
        i = j
    return chunks


def _batches(n):
    # output DMA batches as group-ranges: the first three groups
    # together, then two-group batches, then the last two groups as
    # separate small batches (they sit on the critical tail).
    batches = [(0, min(3, n))]
    i = batches[-1][1]
    while i < n - 3:
        batches.append((i, min(i + 2, n - 3)))
        i = batches[-1][1]
    for j in range(i, n):
        batches.append((j, j + 1))
    return batches


# groups whose BOTH relus run on ACT: the two PSUM-capable engines are
# unequal (ACT ~1.09ns/col vs DVE ~1.24ns/col), so strict alternation
# leaves ACT idle while DVE paces the pipeline; shifting one mid-stream
# group entirely to ACT balances their totals.
BOTH_ACT = {7}
N_WARM_MM = 6   # dummy matmuls that heat the PE HAM during the DMA wait
WARM_W = 512    # width of each dummy matmul


def build_nc(blocks):
    f32 = mybir.dt.float32
    bf16 = mybir.dt.bfloat16
    nc = bacc.Bacc("TRN2", target_bir_lowering=False, debug=False,
                   num_devices=N_CORES)

    cols = blocks * P
    groups = _groups(cols)
    goff = np.concatenate([[0], np.cumsum(groups)]).tolist()
    assert goff[-1] == cols, (goff[-1], cols)
    n_g = len(groups)
    chunks = _chunks(n_g)
    batches = _batches(n_g)

    tsl_t = nc.dram_tensor("tslice", [P, HDR + cols], bf16,
                           kind="ExternalInput").ap()
    out_t = nc.dram_tensor("out", [P, cols], bf16,
                           kind="ExternalOutput").ap()

    group_chunk = {}      # group -> chunk idx
    for ci, (a, b) in enumerate(chunks):
        for g in range(a, b):
            group_chunk[g] = ci

    group_batch = {}      # group -> (batch idx, last-in-batch?)
    for bi, (a, b) in enumerate(batches):
        for g in range(a, b):
            group_batch[g] = (bi, g == b - 1)

    with TileContext(nc) as tc:
        with (
            tc.tile_pool(name="const", bufs=1) as cpool,
            tc.tile_pool(name="winp", bufs=1) as gpool,
            tc.tile_pool(name="hT", bufs=6) as hpool,
            tc.tile_pool(name="outs", bufs=1) as opool,
            tc.tile_pool(name="psum", bufs=4, space="PSUM") as ppool,
        ):
            # pre-warm the Scalar engine's activation table (the lazy
            # ACT_TABLE_LOAD otherwise lands on the first relu's
            # critical path, costing ~1.3us)
            warm = cpool.tile([P, 1], f32, tag="warm")
            nc.vector.memset(warm[:], 0.0)
            nc.scalar.activation(warm[:], warm[:],
                                 mybir.ActivationFunctionType.Relu, bias=0.0)

            # zero tile feeding the PE warm-up matmuls below
            zdum = cpool.tile([P, WARM_W], bf16, tag="zdum")
            nc.vector.memset(zdum[:], 0.0)

            # pre-warm the GpSimd HWDGE queue: the first DMA on a
            # queue pays a ~0.6us one-time config; a 1-col dummy load
            # moves that off batch 0's flush path.
            scratch = cpool.tile([P, 1], bf16, tag="qwarm")
            nc.gpsimd.dma_start(out=scratch[:], in_=tsl_t[:, 0:1])

            # the whole input window fits in SBUF: queue every chunk's
            # DMA upfront on the Sync queue so chunks stream back-to-
            # back in consumption order at the full 16-engine rate.
            win_tiles = []
            for ci, (a, b) in enumerate(chunks):
                lo = goff[a] + (0 if ci == 0 else HDR)
                cw = goff[b] - goff[a] + (HDR if ci == 0 else 0)
                win = gpool.tile([P, cw], bf16, tag=f"win{ci}")
                nc.sync.dma_start(out=win[:], in_=tsl_t[:, lo : lo + cw])
                win_tiles.append(win)

            hdr = win_tiles[0]
            w1t_sb = hdr[:, 0:D]
            w2t_sb = hdr[:, D : 2 * D]
            b12_sb = hdr[:, 2 * D : 2 * D + 4].bitcast(f32)
            b1_sb = b12_sb[:, 0:1]
            b2_sb = b12_sb[:, 1:2]

            def win_ap(g):
                ci = group_chunk[g]
                a, _ = chunks[ci]
                off = goff[g] - goff[a] + (HDR if ci == 0 else 0)
                return win_tiles[ci][:, off : off + groups[g]]

            # one SBUF tile per output batch
            out_tiles = []
            for bi, (a, b) in enumerate(batches):
                bw = goff[b] - goff[a]
                out_tiles.append(opool.tile([P, bw], bf16,
                                            name=f"outsb{bi}",
                                            tag=f"out{bi}"))

            def relu_bias(out_ap, in_ap, bias_sb, on_act):
                if on_act:
                    nc.scalar.activation(out_ap, in_ap,
                                         mybir.ActivationFunctionType.Relu,
                                         bias=bias_sb)
                else:
                    nc.vector.tensor_scalar(
                        out=out_ap, in0=in_ap, scalar1=bias_sb,
                        scalar2=0.0, op0=mybir.AluOpType.add,
                        op1=mybir.AluOpType.max)

            ps = {}
            hT = {}

            def mm1(g):
                gw = groups[g]
                src = win_ap(g)
                t = ppool.tile([P, GW], f32, tag="ps")
                for s in range(0, gw, MMW):
                    w = min(MMW, gw - s)
                    nc.tensor.matmul(out=t[:, s : s + w], lhsT=w1t_sb,
                                     rhs=src[:, s : s + w],
                                     start=True, stop=True)
                ps[g] = t

            def rh(g):
                gw = groups[g]
                t = hpool.tile([P, GW], bf16, tag="hT")
                relu_bias(t[:, :gw], ps[g][:, :gw], b1_sb,
                          on_act=(g % 2 == 0 or g in BOTH_ACT))
                hT[g] = t

            def mm2(g):
                # overwrites the group's own h PSUM tile (free once
                # relu_h has read it)
                gw = groups[g]
                src = hT.pop(g)
                t = ps[g]
                for s in range(0, gw, MMW):
                    w = min(MMW, gw - s)
                    nc.tensor.matmul(out=t[:, s : s + w], lhsT=w2t_sb,
                                     rhs=src[:, s : s + w],
                                     start=True, stop=True)

            def ro(g):
                gw = groups[g]
                bi, last = group_batch[g]
                a, b = batches[bi]
                boff = goff[g] - goff[a]
                out_sb = out_tiles[bi]
                relu_bias(out_sb[:, boff : boff + gw], ps.pop(g)[:, :gw],
                          b2_sb, on_act=(g % 2 == 1 or g in BOTH_ACT))
                if last:
                    bw = goff[b] - goff[a]
                    bstart = goff[a]
                    if bi == len(batches) - 1:
                        # FINAL batch: partition-split across both
                        # queues; with the tail routing below, each
                        # queue has at most one earlier tail trigger,
                        # so the halves issue with no queue backlog
                        nc.gpsimd.dma_start(
                            out=out_t[0:64, bstart : bstart + bw],
                            in_=out_sb[0:64, :])
                        nc.sync.dma_start(
                            out=out_t[64:128, bstart : bstart + bw],
                            in_=out_sb[64:128, :])
                    elif bi == len(batches) - 2:
                        # second-to-last: whole on Sync (idle since
                        # the input triggers) so GpSimd's queue stays
                        # clear for the final batch's half
                        nc.sync.dma_start(
                            out=out_t[:, bstart : bstart + bw],
                            in_=out_sb[:])
                    else:
                        nc.gpsimd.dma_start(
                            out=out_t[:, bstart : bstart + bw],
                            in_=out_sb[:])

            # PE warm-up: the HAM clock gate keeps a cold PE at
            # 1.2GHz until it has seen ~3.4us of sustained activity.
            # Zero-input dummy matmuls during the otherwise-idle
            # chunk-0 DMA wait flip it to 2.4GHz right as real work
            # arrives (cold 512-col matmuls cost 630ns vs 215 warm).
            wps = ppool.tile([P, GW], f32, tag="ps")
            for _ in range(N_WARM_MM):
                nc.tensor.matmul(out=wps[:, 0:WARM_W], lhsT=zdum[:, 0:D],
                                 rhs=zdum[:], start=True, stop=True)

            # software pipeline: mm1 runs 3 groups ahead through the
            # 4-deep PSUM rotation so relu latency never starves the
            # PE or lets the DMA bus idle.
            LOOK = 3
            for g in range(min(LOOK, n_g)):
                mm1(g)
            if n_g:
                rh(0)
            for g in range(n_g):
                if g + LOOK < n_g:
                    mm1(g + LOOK)
                if g + 1 < n_g:
                    rh(g + 1)
                mm2(g)
                ro(g)

    nc.compile()
    return nc


_CACHED_NC = {}
LAST_RESULTS = None


def _get_nc(blocks):
    if blocks not in _CACHED_NC:
        _CACHED_NC[blocks] = build_nc(blocks)
    return _CACHED_NC[blocks]


def _run(in_maps):
    import os

    trace = os.environ.get("BASS_KERNEL_TRACE") == "1"
    if trace:
        try:  # tracing needs the NTFF hook; degrade silently without it
            import antenv.axon_hooks  # noqa: F401
        except ImportError:
            trace = False
    blocks = (in_maps[0]["tslice"].shape[1] - HDR) // P
    res = run_bass_kernel_spmd(_get_nc(blocks), in_maps,
                               core_ids=list(range(N_CORES)), trace=trace)
    global LAST_RESULTS
    LAST_RESULTS = res
    return res


def _make_header(w1, b1, w2, b2):
    wb = np.concatenate([w1.T, w2.T], axis=1).astype(BF16)    # [128, 256]
    b12 = np.stack([b1, b2], axis=1).astype(np.float32)       # [128, 2]
    b12_bf = b12.view(BF16)                                   # [128, 4] raw
    pad = np.zeros((P, HDR - 2 * D - 4), dtype=BF16)
    return np.concatenate([wb, b12_bf, pad], axis=1)          # [128, HDR]


def kernel(nodes, c2e_weight, w1, b1, w2, b2):
    nodes = np.asarray(nodes).astype(np.int64)
    c2e_weight = np.asarray(c2e_weight, dtype=np.float32)
    w1 = np.asarray(w1, dtype=np.float32)
    b1 = np.asarray(b1, dtype=np.float32)
    w2 = np.asarray(w2, dtype=np.float32)
    b2 = np.asarray(b2, dtype=np.float32)

    vocab = c2e_weight.shape[0]
    assert vocab == VOCAB, vocab

    tableT = np.ascontiguousarray(c2e_weight.T).astype(BF16)  # [128, V]
    header = _make_header(w1, b1, w2, b2)

    # Compacted path: only vocab rows actually referenced by `nodes`
    # are pushed through the MLP (~86.4% of the vocab for uniform
    # ids).  Falls back to a wider layout / transforming the full
    # table when the unique count doesn't fit.
    uniq, inv = np.unique(nodes, return_inverse=True)
    u = len(uniq)
    for cblocks in (CBLOCKS, CBLOCKS2):
        crows = cblocks * P
        if u > N_CORES * crows:
            continue
        tc = tableT[:, uniq]                              # [128, U] gather
        if u < N_CORES * crows:
            tc = np.concatenate(
                [tc, np.zeros((P, N_CORES * crows - u), dtype=BF16)],
                axis=1)
        in_maps = [{
            "tslice": np.ascontiguousarray(np.concatenate(
                [header, tc[:, i * crows : (i + 1) * crows]], axis=1)),
        } for i in range(N_CORES)]
        res = _run(in_maps)
        t2c = np.empty((u, D), dtype=np.float32)
        for i in range(N_CORES):
            lo = i * crows
            hi = min(lo + crows, u)
            if lo >= u:
                break
            dense = res.results[i]["out"]                 # [128, crows]
            t2c[lo:hi] = dense[:, : hi - lo].T
        return t2c[inv]

    # Full-table fallback: vocab-range sharding, T2[v] for every v
    rows = FBLOCKS * P
    starts = []
    in_maps = []
    for i in range(N_CORES):
        start = min(i * RANGE, vocab - rows)
        starts.append(start)
        in_maps.append({
            "tslice": np.ascontiguousarray(np.concatenate(
                [header, tableT[:, start : start + rows]], axis=1)),
        })
    res = _run(in_maps)
    t2 = np.empty((vocab, D), dtype=np.float32)
    for i in range(N_CORES):
        dense = res.results[i]["out"]                    # [128, rows] (k, r)
        lo = i * RANGE
        hi = min((i + 1) * RANGE, vocab)
        t2[lo:hi] = dense[:, lo - starts[i] : hi - starts[i]].T

    return t2[nodes]


# revision 47
# speedup vs baseline: 1.0765x; 1.0618x over previous
"""Trainium2 Bass kernel for the Context Encoder problem:

    ce  = c2e_weight[nodes]            # [N, 128] embedding gather
    h   = relu(ce @ w1.T + b1)         # [N, 128]
    out = relu(h @ w2.T + b2)          # [N, 128]

Strategy (8 NeuronCores, unique-row compaction):
  200000 node ids hit ~86.4k of the 100k vocab rows, so transforming
  each UNIQUE row once is cheaper than gathering per-node rows.  The
  host maps node positions to unique rows (out = T2c[inv]) as the
  unshard step.  Core i streams its host-pre-transposed (d-major)
  slice of the compacted table [128, ~10880] and computes
  T2 = relu(relu(win@w1.T+b1)@w2.T+b2) column-by-column.

  Perf structure (memory regime, ~26GB/s x 16 DMA engines/core):
  - bf16 on the wire and through the PE: halves HBM traffic; the warm
    PE streams 512-col bf16 matmuls every ~215ns with LDWEIGHTS hidden
    by its 64-deep reorder window, so w1/w2 alternation is free.  PSUM
    stays f32.
  - 1024-col groups (2 PSUM banks): relu+bias is ONE fused op per
    group per layer, alternated ACT/DVE (the only engines with PSUM
    ports; ~1.11/1.27us per op, which sets the compute pace).  Small
    first groups (128, 512) prime the pipeline; small last groups
    (512, 512) shorten the drain.
  - mm1 is emitted 3 groups ahead through a 4-deep PSUM rotation
    (8 banks), so the mm1 -> relu_h -> mm2 -> relu_o chain never
    starves the PE; mm2 overwrites its own group's PSUM tile (free
    once relu_h read it) - no WAR stalls.
  - ALL input chunks ride the Sync HWDGE queue (a queue's chunks
    transfer serially at full 16-engine rate, so in-order chunk
    completion is earliest), one chunk per group so a group's matmul
    never waits on a later group's data.  Output batches of ~2 groups
    flush on the otherwise-idle GpSimd queue as soon as their relu_o
    completes, so the out-stream overlaps the in-stream and compute
    window; the small tail batches are partition-split across both
    queues to shorten the critical tail.  Neither relu engine ever
    issues a DMA.
  - Zero-input dummy matmuls during the initial DMA wait heat the
    PE's HAM clock gate (cold = 1.2GHz, warm = 2.4GHz) so real
    matmuls run warm from the start.
"""

import sys

for _p in ("/opt/trn_rl_repo",):
    if _p not in sys.path:
        sys.path.insert(0, _p)

import ml_dtypes
import numpy as np

import concourse.bass as bass
import concourse.mybir as mybir
from concourse import bacc
from concourse.bass_utils import run_bass_kernel_spmd
from concourse.tile import TileContext

P = 128
D = 128
N_CORES = 8
VOCAB = 100000
RANGE = VOCAB // N_CORES   # full-table fallback: vocab rows per core
FBLOCKS = 98               # full-table fallback: 12544 rows/core
CBLOCKS = 85               # compacted (unique-rows) path: 10880 rows/core
CBLOCKS2 = 88              # compacted fallback if uniques don't fit 85
MMW = 512                  # matmul free width (1 PSUM bank)
GW = 1024                  # relu group width (2 PSUM banks)
HDR = 264                  # header cols prepended to chunk 0

BF16 = ml_dtypes.bfloat16


def _groups(cols):
    # [128, 512] primer groups, full 1024-col groups mid-stream, and
    # two ~half groups at the end to shorten the pipeline drain.
    rem = cols - 640
    k, r = divmod(rem, GW)
    tail = GW + r
    a = ((tail // 2) + P - 1) // P * P
    return [P, MMW] + [GW] * (k - 1) + [a, tail - a]


def _chunks(n):
    # input DMA chunks as group-ranges: singles for the three primer
    # groups (earliest possible compute start), then PAIRS.  Groups
    # arrive at ~1.0us/group with single-group chunks - exactly the
    # relu consumption pace, so every per-chunk ramp hiccup (~0.3us
    # desc-fetch penalty) becomes a pipeline bubble; pairing halves
    # the per-group overhead and builds ~0.8us of arrival slack.
    k = min(3, n)
    chunks = [(i, i + 1) for i in range(k)]
    i = k
    while i < n:
        j = min(i + 2, n)
        if n - j == 1:
            j = n
        chunks.append((i, j))
        i = j
    return chunks


def _batches(n):
    # output DMA batches as group-ranges: the first three groups
    # together, then two-group batches, then the last two groups as
    # separate small batches (they sit on the critical tail).
    batches = [(0, min(3, n))]
    i = batches[-1][1]
    while i < n - 3:
        batches.append((i, min(i + 2, n - 3)))
        i = batches[-1][1]
    for j in range(i, n):
        batches.append((j, j + 1))
    return batches


# groups whose BOTH relus run on ACT: the two PSUM-capable engines are
# unequal (ACT ~1.09ns/col vs DVE ~1.24ns/col), so strict alternation
# leaves ACT idle while DVE paces the pipeline; shifting one mid-stream
# group entirely to ACT balances their totals.
BOTH_ACT = {7}
N_WARM_MM = 6   # dummy matmuls that heat the PE HAM during the DMA wait
WARM_W = 512    # width of each dummy matmul


def build_nc(blocks):
    f32 = mybir.dt.float32
    bf16 = mybir.dt.bfloat16
    nc = bacc.Bacc("TRN2", target_bir_lowering=False, debug=False,
                   num_devices=N_CORES)

    cols = blocks * P
    groups = _groups(cols)
    goff = np.concatenate([[0], np.cumsum(groups)]).tolist()
    assert goff[-1] == cols, (goff[-1], cols)
    n_g = len(groups)
    chunks = _chunks(n_g)
    batches = _batches(n_g)

    tsl_t = nc.dram_tensor("tslice", [P, HDR + cols], bf16,
                           kind="ExternalInput").ap()
    out_t = nc.dram_tensor("out", [P, cols], bf16,
                           kind="ExternalOutput").ap()

    group_chunk = {}      # group -> chunk idx
    for ci, (a, b) in enumerate(chunks):
        for g in range(a, b):
            group_chunk[g] = ci

    group_batch = {}      # group -> (batch idx, last-in-batch?)
    for bi, (a, b) in enumerate(batches):
        for g in range(a, b):
            group_batch[g] = (bi, g == b - 1)

    with TileContext(nc) as tc:
        with (
            tc.tile_pool(name="const", bufs=1) as cpool,
            tc.tile_pool(name="winp", bufs=1) as gpool,
            tc.tile_pool(name="hT", bufs=6) as hpool,
            tc.tile_pool(name="outs", bufs=1) as opool,
            tc.tile_pool(name="psum", bufs=4, space="PSUM") as ppool,
        ):
            # pre-warm the Scalar engine's activation table (the lazy
            # ACT_TABLE_LOAD otherwise lands on the first relu's
            # critical path, costing ~1.3us)
            warm = cpool.tile([P, 1], f32, tag="warm")
            nc.vector.memset(warm[:], 0.0)
            nc.scalar.activation(warm[:], warm[:],
                                 mybir.ActivationFunctionType.Relu, bias=0.0)

            # zero tile feeding the PE warm-up matmuls below
            zdum = cpool.tile([P, WARM_W], bf16, tag="zdum")
            nc.vector.memset(zdum[:], 0.0)

            # pre-warm the GpSimd HWDGE queue: the first DMA on a
            # queue pays a ~0.6us one-time config; a 1-col dummy load
            # moves that off batch 0's flush path.
            scratch = cpool.tile([P, 1], bf16, tag="qwarm")
            nc.gpsimd.dma_start(out=scratch[:], in_=tsl_t[:, 0:1])

            # the whole input window fits in SBUF: queue every chunk's
            # DMA upfront on the Sync queue so chunks stream back-to-
            # back in consumption order at the full 16-engine rate.
            win_tiles = []
            for ci, (a, b) in enumerate(chunks):
                lo = goff[a] + (0 if ci == 0 else HDR)
                cw = goff[b] - goff[a] + (HDR if ci == 0 else 0)
                win = gpool.tile([P, cw], bf16, tag=f"win{ci}")
                nc.sync.dma_start(out=win[:], in_=tsl_t[:, lo : lo + cw])
                win_tiles.append(win)

            hdr = win_tiles[0]
            w1t_sb = hdr[:, 0:D]
            w2t_sb = hdr[:, D : 2 * D]
            b12_sb = hdr[:, 2 * D : 2 * D + 4].bitcast(f32)
            b1_sb = b12_sb[:, 0:1]
            b2_sb = b12_sb[:, 1:2]

            def win_ap(g):
                ci = group_chunk[g]
                a, _ = chunks[ci]
                off = goff[g] - goff[a] + (HDR if ci == 0 else 0)
                return win_tiles[ci][:, off : off + groups[g]]

            # one SBUF tile per output batch
            out_tiles = []
            for bi, (a, b) in enumerate(batches):
                bw = goff[b] - goff[a]
                out_tiles.append(opool.tile([P, bw], bf16,
                                            name=f"outsb{bi}",
                                            tag=f"out{bi}"))

            def relu_bias(out_ap, in_ap, bias_sb, on_act):
                if on_act:
                    nc.scalar.activation(out_ap, in_ap,
                                         mybir.ActivationFunctionType.Relu,
                                         bias=bias_sb)
                else:
                    nc.vector.tensor_scalar(
                        out=out_ap, in0=in_ap, scalar1=bias_sb,
                        scalar2=0.0, op0=mybir.AluOpType.add,
                        op1=mybir.AluOpType.max)

            ps = {}
            hT = {}

            def mm1(g):
                gw = groups[g]
                src = win_ap(g)
                t = ppool.tile([P, GW], f32, tag="ps")
                for s in range(0, gw, MMW):
                    w = min(MMW, gw - s)
                    nc.tensor.matmul(out=t[:, s : s + w], lhsT=w1t_sb,
                                     rhs=src[:, s : s + w],
                                     start=True, stop=True)
                ps[g] = t

            def rh(g):
                gw = groups[g]
                t = hpool.tile([P, GW], bf16, tag="hT")
                relu_bias(t[:, :gw], ps[g][:, :gw], b1_sb,
                          on_act=(g % 2 == 0 or g in BOTH_ACT))
                hT[g] = t

            def mm2(g):
                # overwrites the group's own h PSUM tile (free once
                # relu_h has read it)
                gw = groups[g]
                src = hT.pop(g)
                t = ps[g]
                for s in range(0, gw, MMW):
                    w = min(MMW, gw - s)
                    nc.tensor.matmul(out=t[:, s : s + w], lhsT=w2t_sb,
                                     rhs=src[:, s : s + w],
                                     start=True, stop=True)

            def ro(g):
                gw = groups[g]
                bi, last = group_batch[g]
                a, b = batches[bi]
                boff = goff[g] - goff[a]
                out_sb = out_tiles[bi]
                relu_bias(out_sb[:, boff : boff + gw], ps.pop(g)[:, :gw],
                          b2_sb, on_act=(g % 2 == 1 or g in BOTH_ACT))
                if last:
                    bw = goff[b] - goff[a]
                    bstart = goff[a]
                    if bi >= len(batches) - 4:
                        # tail batches: partition-split across both
                        # queues so their transfers and desc-gen run
                        # in parallel (they sit on the critical tail;
                        # Sync's input triggers are long done)
                        nc.gpsimd.dma_start(
                            out=out_t[0:64, bstart : bstart + bw],
                            in_=out_sb[0:64, :])
                        nc.sync.dma_start(
                            out=out_t[64:128, bstart : bstart + bw],
                            in_=out_sb[64:128, :])
                    else:
                        nc.gpsimd.dma_start(
                            out=out_t[:, bstart : bstart + bw],
                            in_=out_sb[:])

            # PE warm-up: the HAM clock gate keeps a cold PE at
            # 1.2GHz until it has seen ~3.4us of sustained activity.
            # Zero-input dummy matmuls during the otherwise-idle
            # chunk-0 DMA wait flip it to 2.4GHz right as real work
            # arrives (cold 512-col matmuls cost 630ns vs 215 warm).
            wps = ppool.tile([P, GW], f32, tag="ps")
            for _ in range(N_WARM_MM):
                nc.tensor.matmul(out=wps[:, 0:WARM_W], lhsT=zdum[:, 0:D],
                                 rhs=zdum[:], start=True, stop=True)

            # software pipeline: mm1 runs 2 groups ahead through the
            # 4-deep PSUM rotation - enough to hide relu latency from
            # the PE, while keeping the lookahead mm1's chunk-arrival
            # wait from blocking ready mm2s behind it in the PE queue
            # (with paired chunks, group g+2's data always lands
            # before mm2(g)'s inputs are ready; g+3's does not).
            LOOK = 2
            for g in range(min(LOOK, n_g)):
                mm1(g)
            if n_g:
                rh(0)
            for g in range(n_g):
                if g + LOOK < n_g:
                    mm1(g + LOOK)
                if g + 1 < n_g:
                    rh(g + 1)
                mm2(g)
                ro(g)

    nc.compile()
    return nc


_CACHED_NC = {}
LAST_RESULTS = None


def _get_nc(blocks):
    if blocks not in _CACHED_NC:
        _CACHED_NC[blocks] = build_nc(blocks)
    return _CACHED_NC[blocks]


def _run(in_maps):
    import os

    trace = os.environ.get("BASS_KERNEL_TRACE") == "1"
    if trace:
        try:  # tracing needs the NTFF hook; degrade silently without it
            import antenv.axon_hooks  # noqa: F401
        except ImportError:
            trace = False
    blocks = (in_maps[0]["tslice"].shape[1] - HDR) // P
    res = run_bass_kernel_spmd(_get_nc(blocks), in_maps,
                               core_ids=list(range(N_CORES)), trace=trace)
    global LAST_RESULTS
    LAST_RESULTS = res
    return res


def _make_header(w1, b1, w2, b2):
    wb = np.concatenate([w1.T, w2.T], axis=1).astype(BF16)    # [128, 256]
    b12 = np.stack([b1, b2], axis=1).astype(np.float32)       # [128, 2]
    b12_bf = b12.view(BF16)                                   # [128, 4] raw
    pad = np.zeros((P, HDR - 2 * D - 4), dtype=BF16)
    return np.concatenate([wb, b12_bf, pad], axis=1)          # [128, HDR]


def kernel(nodes, c2e_weight, w1, b1, w2, b2):
    nodes = np.asarray(nodes).astype(np.int64)
    c2e_weight = np.asarray(c2e_weight, dtype=np.float32)
    w1 = np.asarray(w1, dtype=np.float32)
    b1 = np.asarray(b1, dtype=np.float32)
    w2 = np.asarray(w2, dtype=np.float32)
    b2 = np.asarray(b2, dtype=np.float32)

    vocab = c2e_weight.shape[0]
    assert vocab == VOCAB, vocab

    tableT = np.ascontiguousarray(c2e_weight.T).astype(BF16)  # [128, V]
    header = _make_header(w1, b1, w2, b2)

    # Compacted path: only vocab rows actually referenced by `nodes`
    # are pushed through the MLP (~86.4% of the vocab for uniform
    # ids).  Falls back to a wider layout / transforming the full
    # table when the unique count doesn't fit.
    uniq, inv = np.unique(nodes, return_inverse=True)
    u = len(uniq)
    for cblocks in (CBLOCKS, CBLOCKS2):
        crows = cblocks * P
        if u > N_CORES * crows:
            continue
        tc = tableT[:, uniq]                              # [128, U] gather
        if u < N_CORES * crows:
            tc = np.concatenate(
                [tc, np.zeros((P, N_CORES * crows - u), dtype=BF16)],
                axis=1)
        in_maps = [{
            "tslice": np.ascontiguousarray(np.concatenate(
                [header, tc[:, i * crows : (i + 1) * crows]], axis=1)),
        } for i in range(N_CORES)]
        res = _run(in_maps)
        t2c = np.empty((u, D), dtype=np.float32)
        for i in range(N_CORES):
            lo = i * crows
            hi = min(lo + crows, u)
            if lo >= u:
                break
            dense = res.results[i]["out"]                 # [128, crows]
            t2c[lo:hi] = dense[:, : hi - lo].T
        return t2c[inv]

    # Full-table fallback: vocab-range sharding, T2[v] for every v
    rows = FBLOCKS * P
    starts = []
    in_maps = []
    for i in range(N_CORES):
        start = min(i * RANGE, vocab - rows)
        starts.append(start)
        in_maps.append({
            "tslice": np.ascontiguousarray(np.concatenate(
                [header, tableT[:, start : start + rows]], axis=1)),
        })
    res = _run(in_maps)
    t2 = np.empty((vocab, D), dtype=np.float32)
    for i in range(N_CORES):
        dense = res.results[i]["out"]                    # [128, rows] (k, r)
        lo = i * RANGE
        hi = min((i + 1) * RANGE, vocab)
        t2[lo:hi] = dense[:, lo - starts[i] : hi - starts[i]].T

    return t2[nodes]
